# revision 2
# baseline (speedup 1.0000x reference)
"""MoE router gate kernel for Trainium2 (Bass/Tile), 8-core data-parallel.

Computes, for x[16384, 7168], weight[256, 7168], bias[256]:
    scores  = sigmoid(x @ weight.T)
    biased  = scores + bias
    indices = top8(biased)                        (descending, int32)
    weights = scores[indices] / sum * 2.5         (float32)

Sharding: data-parallel over tokens (2048 tokens/core), weight/bias
replicated.  Host pre-arranges x into a transposed tiled layout
[16, 128(d_in), 56(d_out), 128(tok)] per core so the contraction dim
lands on SBUF partitions with fully-contiguous DMAs and no on-device
transposes.
"""

import os
from concurrent.futures import ThreadPoolExecutor

import numpy as np

TOKENS = 16384
DIM = 7168
NEXP = 256
TOPK = 8
ROUTE_SCALE = 2.5
NCORES = 8
TPC = TOKENS // NCORES          # tokens per core: 2048
P = 128                         # partitions / tile height
NTILES = TPC // P               # 16 token tiles per core
KC = DIM // P                   # 56 contraction chunks

# Matmul input precision: "fp32" (exact, 4 cyc/row) or "f32r" (1 cyc/row)
MM_DTYPE = os.environ.get("GATE_MM_DTYPE", "fp32")


def _build_program():
    import concourse.bacc as bacc
    import concourse.mybir as mybir
    import concourse.tile as tile

    f32 = mybir.dt.float32
    u32 = mybir.dt.uint32
    mm_dt = {"fp32": mybir.dt.float32, "f32r": mybir.dt.float32r}[MM_DTYPE]

    nc = bacc.Bacc(
        "TRN2",
        target_bir_lowering=False,
        debug=False,
        enable_asserts=False,
        num_devices=NCORES,
    )

    xt_d = nc.dram_tensor("xt", [NTILES, P, KC, P], f32, kind="ExternalInput").ap()
    wt_d = nc.dram_tensor("wt", [P, KC, NEXP], f32, kind="ExternalInput").ap()
    bb_d = nc.dram_tensor("bb", [P, NEXP], f32, kind="ExternalInput").ap()
    ow_d = nc.dram_tensor("out_w", [NTILES, P, TOPK], f32, kind="ExternalOutput").ap()
    oi_d = nc.dram_tensor("out_i", [NTILES, P, TOPK], u32, kind="ExternalOutput").ap()

    with tile.TileContext(nc) as tc:
        with (
            tc.tile_pool(name="const", bufs=1) as const_pool,
            tc.tile_pool(name="xin", bufs=3) as x_pool,
            tc.tile_pool(name="psum", bufs=4, space="PSUM") as ps_pool,
            tc.tile_pool(name="epi", bufs=3) as ep_pool,
        ):
            wt_sb = const_pool.tile([P, KC, NEXP], mm_dt)
            nc.sync.dma_start(wt_sb[:], wt_d)
            bb_sb = const_pool.tile([P, NEXP], f32)
            nc.sync.dma_start(bb_sb[:], bb_d)

            for b in range(NTILES):
                xt_sb = x_pool.tile([P, KC, P], mm_dt, tag="xt")
                nc.sync.dma_start(xt_sb[:], xt_d[b])

                ps = ps_pool.tile([P, NEXP], f32, tag="ps")
                for k in range(KC):
                    nc.tensor.matmul(
                        ps[:],
                        xt_sb[:, k, :],
                        wt_sb[:, k, :],
                        start=(k == 0),
                        stop=(k == KC - 1),
                    )

                sig = ep_pool.tile([P, NEXP], f32, tag="sig")
                nc.scalar.activation(
                    sig[:], ps[:], mybir.ActivationFunctionType.Sigmoid
                )

                biased = ep_pool.tile([P, NEXP], f32, tag="biased")
                nc.vector.tensor_add(biased[:], sig[:], bb_sb[:])

                max8 = ep_pool.tile([P, TOPK], f32, tag="max8")
                nc.vector.max(out=max8[:], in_=biased[:])
                idx = ep_pool.tile([P, TOPK], u32, tag="idx")
                nc.vector.max_index(out=idx[:], in_max=max8[:], in_values=biased[:])

                # Gather original sigmoid scores at the selected experts:
                # sel[:, j] = sum_e (biased[:, e] == max8[:, j]) * sig[:, e]
                sel = ep_pool.tile([P, TOPK], f32, tag="sel")
                scratch = ep_pool.tile([P, NEXP], f32, tag="scratch")
                for j in range(TOPK):
                    nc.vector.scalar_tensor_tensor(
                        out=scratch[:],
                        in0=biased[:],
                        scalar=max8[:, j : j + 1],
                        in1=sig[:],
                        op0=mybir.AluOpType.is_equal,
                        op1=mybir.AluOpType.mult,
                        accum_out=sel[:, j : j + 1],
                    )

                ssum = ep_pool.tile([P, 1], f32, tag="ssum")
                nc.vector.tensor_reduce(
                    ssum[:], sel[:], axis=mybir.AxisListType.X, op=mybir.AluOpType.add
                )
                rec = ep_pool.tile([P, 1], f32, tag="rec")
                nc.vector.reciprocal(rec[:], ssum[:])

                wout = ep_pool.tile([P, TOPK], f32, tag="wout")
                nc.vector.tensor_scalar(
                    wout[:],
                    sel[:],
                    rec[:],
                    ROUTE_SCALE,
                    op0=mybir.AluOpType.mult,
                    op1=mybir.AluOpType.mult,
                )

                nc.sync.dma_start(ow_d[b], wout[:])
                nc.sync.dma_start(oi_d[b], idx[:])

    nc.compile()
    return nc


def _prep_core_inputs(x_shard, wt, bb):
    # x_shard [2048, 7168] -> [16, 128(tok), 56(d_out), 128(d_in)]
    #                      -> [16, 128(d_in), 56(d_out), 128(tok)]
    xt = np.ascontiguousarray(
        x_shard.reshape(NTILES, P, KC, P).transpose(0, 3, 2, 1)
    )
    return {"xt": xt, "wt": wt, "bb": bb}


def _prep_all(x, w, bias):
    # weight [256, 7168] -> [128(d_in), 56(d_out), 256(exp)]
    wt = np.ascontiguousarray(w.reshape(NEXP, KC, P).transpose(2, 1, 0))
    bb = np.ascontiguousarray(np.broadcast_to(bias, (P, NEXP)))

    with ThreadPoolExecutor(NCORES) as pool:
        return list(
            pool.map(
                lambda c: _prep_core_inputs(x[c * TPC : (c + 1) * TPC], wt, bb),
                range(NCORES),
            )
        )


def _collect(results):
    weights = np.concatenate(
        [r["out_w"].reshape(TPC, TOPK) for r in results], axis=0
    ).astype(np.float32)
    indices = np.concatenate(
        [r["out_i"].reshape(TPC, TOPK) for r in results], axis=0
    ).astype(np.int32)
    return weights, indices


def kernel(**inputs):
    from concourse.bass_utils import run_bass_kernel_spmd

    x = np.ascontiguousarray(np.asarray(inputs["x"], dtype=np.float32))
    w = np.ascontiguousarray(np.asarray(inputs["weight"], dtype=np.float32))
    bias = np.asarray(inputs["bias"], dtype=np.float32)

    in_maps = _prep_all(x, w, bias)
    nc = _build_program()
    res = run_bass_kernel_spmd(nc, in_maps, core_ids=list(range(NCORES)), trace=False)
    return _collect(res.results)


# revision 6
# speedup vs baseline: 10.2024x; 10.2024x over previous
"""MoE router gate kernel for Trainium2 (Bass/Tile), 8-core data-parallel.

Computes, for x[16384, 7168], weight[256, 7168], bias[256]:
    scores  = sigmoid(x @ weight.T)
    biased  = scores + bias
    indices = top8(biased)                        (descending, int32)
    weights = scores[indices] / sum * 2.5         (float32)

Sharding: data-parallel over tokens (2048 tokens/core), weight/bias
replicated.  Host pre-arranges x into a transposed tiled layout
[16, 128(d_in), 56(d_out), 128(tok)] per core so the contraction dim
lands on SBUF partitions with fully-contiguous DMAs and no on-device
transposes.
"""

import os
from concurrent.futures import ThreadPoolExecutor

import numpy as np

TOKENS = 16384
DIM = 7168
NEXP = 256
TOPK = 8
ROUTE_SCALE = 2.5
NCORES = 8
TPC = TOKENS // NCORES          # tokens per core: 2048
P = 128                         # partitions / tile height
NTILES = TPC // P               # 16 token tiles per core
KC = DIM // P                   # 56 contraction chunks

# Matmul input precision:
#   "fp32"   exact, 4 cyc/row
#   "f32r"   1 cyc/row at N>=256, reduced-precision multiply
#   "fp16x2" hi/lo fp16 split, 3 matmuls at 1 cyc/row, ~fp32 precision
MM_DTYPE = os.environ.get("GATE_MM_DTYPE", "fp32")
X_SCALE = 16.0   # fp16x2: keep x_lo out of fp16-denormal range
W_SCALE = 64.0   # fp16x2: keep w_lo out of fp16-denormal range


def _build_program(reps=1):
    import concourse.bacc as bacc
    import concourse.mybir as mybir
    import concourse.tile as tile

    f32 = mybir.dt.float32
    f16 = mybir.dt.float16
    u32 = mybir.dt.uint32
    split = MM_DTYPE == "fp16x2"
    mm_dt = f16 if split else {
        "fp32": mybir.dt.float32,
        "f32r": mybir.dt.float32r,
    }[MM_DTYPE]
    sig_scale = 1.0 / (X_SCALE * W_SCALE) if split else 1.0

    nc = bacc.Bacc(
        "TRN2",
        target_bir_lowering=False,
        debug=False,
        enable_asserts=False,
        num_devices=NCORES,
    )

    if split:
        xh_d = nc.dram_tensor("xh", [NTILES, P, KC, P], f16, kind="ExternalInput").ap()
        xl_d = nc.dram_tensor("xl", [NTILES, P, KC, P], f16, kind="ExternalInput").ap()
        wh_d = nc.dram_tensor("wh", [P, KC, NEXP], f16, kind="ExternalInput").ap()
        wl_d = nc.dram_tensor("wl", [P, KC, NEXP], f16, kind="ExternalInput").ap()
    else:
        xt_d = nc.dram_tensor("xt", [NTILES, P, KC, P], f32, kind="ExternalInput").ap()
        wt_d = nc.dram_tensor("wt", [P, KC, NEXP], f32, kind="ExternalInput").ap()
    bb_d = nc.dram_tensor("bb", [P, NEXP], f32, kind="ExternalInput").ap()
    ow_d = nc.dram_tensor("out_w", [NTILES, P, TOPK], f32, kind="ExternalOutput").ap()
    oi_d = nc.dram_tensor("out_i", [NTILES, P, TOPK], u32, kind="ExternalOutput").ap()

    with tile.TileContext(nc) as tc:
        with (
            tc.tile_pool(name="const", bufs=1) as const_pool,
            tc.tile_pool(name="xin", bufs=3) as x_pool,
            tc.tile_pool(name="psum", bufs=4, space="PSUM") as ps_pool,
            tc.tile_pool(name="epi", bufs=3) as ep_pool,
        ):
            if split:
                wh_sb = const_pool.tile([P, KC, NEXP], f16)
                nc.sync.dma_start(wh_sb[:], wh_d)
                wl_sb = const_pool.tile([P, KC, NEXP], f16)
                nc.sync.dma_start(wl_sb[:], wl_d)
            else:
                wt_sb = const_pool.tile([P, KC, NEXP], mm_dt)
                nc.sync.dma_start(wt_sb[:], wt_d)
            bb_sb = const_pool.tile([P, NEXP], f32)
            nc.sync.dma_start(bb_sb[:], bb_d)

            for b in [b for _ in range(reps) for b in range(NTILES)]:
                ps = ps_pool.tile([P, NEXP], f32, tag="ps")
                if split:
                    xh_sb = x_pool.tile([P, KC, P], f16, tag="xh")
                    nc.sync.dma_start(xh_sb[:], xh_d[b])
                    xl_sb = x_pool.tile([P, KC, P], f16, tag="xl")
                    nc.sync.dma_start(xl_sb[:], xl_d[b])
                    n_acc = 3 * KC
                    i = 0
                    for k in range(KC):
                        for lhs, rhs in (
                            (xh_sb, wh_sb),
                            (xh_sb, wl_sb),
                            (xl_sb, wh_sb),
                        ):
                            nc.tensor.matmul(
                                ps[:],
                                lhs[:, k, :],
                                rhs[:, k, :],
                                start=(i == 0),
                                stop=(i == n_acc - 1),
                            )
                            i += 1
                else:
                    xt_sb = x_pool.tile([P, KC, P], mm_dt, tag="xt")
                    nc.sync.dma_start(xt_sb[:], xt_d[b])
                    for k in range(KC):
                        nc.tensor.matmul(
                            ps[:],
                            xt_sb[:, k, :],
                            wt_sb[:, k, :],
                            start=(k == 0),
                            stop=(k == KC - 1),
                        )

                sig = ep_pool.tile([P, NEXP], f32, tag="sig")
                nc.scalar.activation(
                    sig[:],
                    ps[:],
                    mybir.ActivationFunctionType.Sigmoid,
                    scale=sig_scale,
                )

                biased = ep_pool.tile([P, NEXP], f32, tag="biased")
                nc.vector.tensor_add(biased[:], sig[:], bb_sb[:])

                max8 = ep_pool.tile([P, TOPK], f32, tag="max8")
                nc.vector.max(out=max8[:], in_=biased[:])
                idx = ep_pool.tile([P, TOPK], u32, tag="idx")
                nc.vector.max_index(out=idx[:], in_max=max8[:], in_values=biased[:])

                # Gather original sigmoid scores at the selected experts:
                # sel[:, j] = sum_e (biased[:, e] == max8[:, j]) * sig[:, e]
                sel = ep_pool.tile([P, TOPK], f32, tag="sel")
                scratch = ep_pool.tile([P, NEXP], f32, tag="scratch")
                for j in range(TOPK):
                    nc.vector.scalar_tensor_tensor(
                        out=scratch[:],
                        in0=biased[:],
                        scalar=max8[:, j : j + 1],
                        in1=sig[:],
                        op0=mybir.AluOpType.is_equal,
                        op1=mybir.AluOpType.mult,
                        accum_out=sel[:, j : j + 1],
                    )

                ssum = ep_pool.tile([P, 1], f32, tag="ssum")
                nc.vector.tensor_reduce(
                    ssum[:], sel[:], axis=mybir.AxisListType.X, op=mybir.AluOpType.add
                )
                rec = ep_pool.tile([P, 1], f32, tag="rec")
                nc.vector.reciprocal(rec[:], ssum[:])

                wout = ep_pool.tile([P, TOPK], f32, tag="wout")
                nc.vector.tensor_scalar(
                    wout[:],
                    sel[:],
                    rec[:],
                    ROUTE_SCALE,
                    op0=mybir.AluOpType.mult,
                    op1=mybir.AluOpType.mult,
                )

                nc.sync.dma_start(ow_d[b], wout[:])
                nc.sync.dma_start(oi_d[b], idx[:])

    nc.compile()
    return nc


def _prep_core_inputs(x_shard, wt, bb):
    # x_shard [2048, 7168] -> [16, 128(tok), 56(d_out), 128(d_in)]
    #                      -> [16, 128(d_in), 56(d_out), 128(tok)]
    xt = np.ascontiguousarray(
        x_shard.reshape(NTILES, P, KC, P).transpose(0, 3, 2, 1)
    )
    return {"xt": xt, "wt": wt, "bb": bb}


def _prep_all(x, w, bias):
    # weight [256, 7168] -> [128(d_in), 56(d_out), 256(exp)]
    wt = np.ascontiguousarray(w.reshape(NEXP, KC, P).transpose(2, 1, 0))
    bb = np.ascontiguousarray(np.broadcast_to(bias, (P, NEXP)))

    with ThreadPoolExecutor(NCORES) as pool:
        return list(
            pool.map(
                lambda c: _prep_core_inputs(x[c * TPC : (c + 1) * TPC], wt, bb),
                range(NCORES),
            )
        )


def _collect(results):
    weights = np.concatenate(
        [r["out_w"].reshape(TPC, TOPK) for r in results], axis=0
    ).astype(np.float32)
    indices = np.concatenate(
        [r["out_i"].reshape(TPC, TOPK) for r in results], axis=0
    ).astype(np.int32)
    return weights, indices


def kernel(**inputs):
    from concourse.bass_utils import run_bass_kernel_spmd

    x = np.ascontiguousarray(np.asarray(inputs["x"], dtype=np.float32))
    w = np.ascontiguousarray(np.asarray(inputs["weight"], dtype=np.float32))
    bias = np.asarray(inputs["bias"], dtype=np.float32)

    in_maps = _prep_all(x, w, bias)
    nc = _build_program()
    res = run_bass_kernel_spmd(nc, in_maps, core_ids=list(range(NCORES)), trace=False)
    return _collect(res.results)


# revision 7
# speedup vs baseline: 16.2057x; 1.5884x over previous
"""MoE router gate kernel for Trainium2 (Bass/Tile), 8-core data-parallel.

Computes, for x[16384, 7168], weight[256, 7168], bias[256]:
    scores  = sigmoid(x @ weight.T)
    biased  = scores + bias
    indices = top8(biased)                        (descending, int32)
    weights = scores[indices] / sum * 2.5         (float32)

Sharding: data-parallel over tokens (2048 tokens/core), weight/bias
replicated.  Host pre-arranges x into a transposed tiled layout
[16, 128(d_in), 56(d_out), 128(tok)] per core so the contraction dim
lands on SBUF partitions with fully-contiguous DMAs and no on-device
transposes.
"""

import os
from concurrent.futures import ThreadPoolExecutor

import numpy as np

TOKENS = 16384
DIM = 7168
NEXP = 256
TOPK = 8
ROUTE_SCALE = 2.5
NCORES = 8
TPC = TOKENS // NCORES          # tokens per core: 2048
P = 128                         # partitions / tile height
NTILES = TPC // P               # 16 token tiles per core
KC = DIM // P                   # 56 contraction chunks

# Matmul input precision:
#   "fp32"   exact, 4 cyc/row
#   "f32r"   1 cyc/row at N>=256, reduced-precision multiply
#   "fp16x2" hi/lo fp16 split, 3 matmuls at 1 cyc/row, ~fp32 precision
MM_DTYPE = os.environ.get("GATE_MM_DTYPE", "fp32")
X_SCALE = 16.0   # fp16x2: keep x_lo out of fp16-denormal range
W_SCALE = 64.0   # fp16x2: keep w_lo out of fp16-denormal range


def _build_program(reps=1):
    import concourse.bacc as bacc
    import concourse.mybir as mybir
    import concourse.tile as tile

    f32 = mybir.dt.float32
    f16 = mybir.dt.float16
    u32 = mybir.dt.uint32
    split = MM_DTYPE == "fp16x2"
    mm_dt = f16 if split else {
        "fp32": mybir.dt.float32,
        "f32r": mybir.dt.float32r,
    }[MM_DTYPE]
    sig_scale = 1.0 / (X_SCALE * W_SCALE) if split else 1.0

    nc = bacc.Bacc(
        "TRN2",
        target_bir_lowering=False,
        debug=False,
        enable_asserts=False,
        num_devices=NCORES,
    )

    if split:
        xh_d = nc.dram_tensor("xh", [NTILES, P, KC, P], f16, kind="ExternalInput").ap()
        xl_d = nc.dram_tensor("xl", [NTILES, P, KC, P], f16, kind="ExternalInput").ap()
        wh_d = nc.dram_tensor("wh", [P, KC, NEXP], f16, kind="ExternalInput").ap()
        wl_d = nc.dram_tensor("wl", [P, KC, NEXP], f16, kind="ExternalInput").ap()
    else:
        xt_d = nc.dram_tensor("xt", [NTILES, P, KC, P], f32, kind="ExternalInput").ap()
        wt_d = nc.dram_tensor("wt", [P, KC, NEXP], f32, kind="ExternalInput").ap()
    bb_d = nc.dram_tensor("bb", [P, NEXP], f32, kind="ExternalInput").ap()
    ow_d = nc.dram_tensor("out_w", [NTILES, P, TOPK], f32, kind="ExternalOutput").ap()
    oi_d = nc.dram_tensor("out_i", [NTILES, P, TOPK], u32, kind="ExternalOutput").ap()

    with tile.TileContext(nc) as tc:
        with (
            tc.tile_pool(name="const", bufs=1) as const_pool,
            tc.tile_pool(name="xin", bufs=3) as x_pool,
            tc.tile_pool(name="psum", bufs=4, space="PSUM") as ps_pool,
            tc.tile_pool(name="epi", bufs=3) as ep_pool,
        ):
            if split:
                wh_sb = const_pool.tile([P, KC, NEXP], f16)
                nc.sync.dma_start(wh_sb[:], wh_d)
                wl_sb = const_pool.tile([P, KC, NEXP], f16)
                nc.sync.dma_start(wl_sb[:], wl_d)
            else:
                wt_sb = const_pool.tile([P, KC, NEXP], mm_dt)
                nc.sync.dma_start(wt_sb[:], wt_d)
            bb_sb = const_pool.tile([P, NEXP], f32)
            nc.sync.dma_start(bb_sb[:], bb_d)

            for b in [b for _ in range(reps) for b in range(NTILES)]:
                ps = ps_pool.tile([P, NEXP], f32, tag="ps")
                if split:
                    xh_sb = x_pool.tile([P, KC, P], f16, tag="xh")
                    nc.sync.dma_start(xh_sb[:], xh_d[b])
                    xl_sb = x_pool.tile([P, KC, P], f16, tag="xl")
                    nc.sync.dma_start(xl_sb[:], xl_d[b])
                    n_acc = 3 * KC
                    i = 0
                    for k in range(KC):
                        for lhs, rhs in (
                            (xh_sb, wh_sb),
                            (xh_sb, wl_sb),
                            (xl_sb, wh_sb),
                        ):
                            nc.tensor.matmul(
                                ps[:],
                                lhs[:, k, :],
                                rhs[:, k, :],
                                start=(i == 0),
                                stop=(i == n_acc - 1),
                            )
                            i += 1
                else:
                    xt_sb = x_pool.tile([P, KC, P], mm_dt, tag="xt")
                    nc.sync.dma_start(xt_sb[:], xt_d[b])
                    for k in range(KC):
                        nc.tensor.matmul(
                            ps[:],
                            xt_sb[:, k, :],
                            wt_sb[:, k, :],
                            start=(k == 0),
                            stop=(k == KC - 1),
                        )

                sig = ep_pool.tile([P, NEXP], f32, tag="sig")
                nc.scalar.activation(
                    sig[:],
                    ps[:],
                    mybir.ActivationFunctionType.Sigmoid,
                    scale=sig_scale,
                )

                biased = ep_pool.tile([P, NEXP], f32, tag="biased")
                nc.vector.tensor_add(biased[:], sig[:], bb_sb[:])

                max8 = ep_pool.tile([P, TOPK], f32, tag="max8")
                nc.vector.max(out=max8[:], in_=biased[:])
                idx = ep_pool.tile([P, TOPK], u32, tag="idx")
                nc.vector.max_index(out=idx[:], in_max=max8[:], in_values=biased[:])

                # Gather original sigmoid scores at the selected experts:
                # sel[:, j] = sum_e (biased[:, e] == max8[:, j]) * sig[:, e]
                sel = ep_pool.tile([P, TOPK], f32, tag="sel")
                scratch = ep_pool.tile([P, NEXP], f32, tag="scratch")
                for j in range(TOPK):
                    nc.vector.scalar_tensor_tensor(
                        out=scratch[:],
                        in0=biased[:],
                        scalar=max8[:, j : j + 1],
                        in1=sig[:],
                        op0=mybir.AluOpType.is_equal,
                        op1=mybir.AluOpType.mult,
                        accum_out=sel[:, j : j + 1],
                    )

                ssum = ep_pool.tile([P, 1], f32, tag="ssum")
                nc.vector.tensor_reduce(
                    ssum[:], sel[:], axis=mybir.AxisListType.X, op=mybir.AluOpType.add
                )
                rec = ep_pool.tile([P, 1], f32, tag="rec")
                nc.vector.reciprocal(rec[:], ssum[:])

                wout = ep_pool.tile([P, TOPK], f32, tag="wout")
                nc.vector.tensor_scalar(
                    wout[:],
                    sel[:],
                    rec[:],
                    ROUTE_SCALE,
                    op0=mybir.AluOpType.mult,
                    op1=mybir.AluOpType.mult,
                )

                nc.sync.dma_start(ow_d[b], wout[:])
                nc.sync.dma_start(oi_d[b], idx[:])

    nc.compile()
    return nc


def _tile_x(x_shard):
    # [2048, D] -> [16, 128(tok), 56(d_out), 128(d_in)] -> [16, 128(d_in), 56, 128(tok)]
    return x_shard.reshape(NTILES, P, KC, P).transpose(0, 3, 2, 1)


def _prep_core_inputs(x_shard, wt, bb):
    if MM_DTYPE == "fp16x2":
        xs = (x_shard * X_SCALE).astype(np.float32)
        xh = xs.astype(np.float16)
        xl = (xs - xh.astype(np.float32)).astype(np.float16)
        return {
            "xh": np.ascontiguousarray(_tile_x(xh)),
            "xl": np.ascontiguousarray(_tile_x(xl)),
            "wh": wt[0],
            "wl": wt[1],
            "bb": bb,
        }
    return {"xt": np.ascontiguousarray(_tile_x(x_shard)), "wt": wt, "bb": bb}


def _prep_all(x, w, bias):
    # weight [256, 7168] -> [128(d_in), 56(d_out), 256(exp)]
    def _tile_w(warr):
        return np.ascontiguousarray(warr.reshape(NEXP, KC, P).transpose(2, 1, 0))

    if MM_DTYPE == "fp16x2":
        ws = (w * W_SCALE).astype(np.float32)
        wh = ws.astype(np.float16)
        wl = (ws - wh.astype(np.float32)).astype(np.float16)
        wt = (_tile_w(wh), _tile_w(wl))
    else:
        wt = _tile_w(w)
    bb = np.ascontiguousarray(np.broadcast_to(bias, (P, NEXP)))

    with ThreadPoolExecutor(NCORES) as pool:
        return list(
            pool.map(
                lambda c: _prep_core_inputs(x[c * TPC : (c + 1) * TPC], wt, bb),
                range(NCORES),
            )
        )


def _collect(results):
    weights = np.concatenate(
        [r["out_w"].reshape(TPC, TOPK) for r in results], axis=0
    ).astype(np.float32)
    indices = np.concatenate(
        [r["out_i"].reshape(TPC, TOPK) for r in results], axis=0
    ).astype(np.int32)
    return weights, indices


def kernel(**inputs):
    from concourse.bass_utils import run_bass_kernel_spmd

    x = np.ascontiguousarray(np.asarray(inputs["x"], dtype=np.float32))
    w = np.ascontiguousarray(np.asarray(inputs["weight"], dtype=np.float32))
    bias = np.asarray(inputs["bias"], dtype=np.float32)

    in_maps = _prep_all(x, w, bias)
    nc = _build_program()
    res = run_bass_kernel_spmd(nc, in_maps, core_ids=list(range(NCORES)), trace=False)
    return _collect(res.results)
